# revision 28
# baseline (speedup 1.0000x reference)
"""Trainium2 Bass kernel for the FFJORD-style ODE function
(3-layer ConcatLinear MLP with softplus + Hutchinson divergence via VJP).

Math (per batch row, feature-major on device):
    X0 = W0z @ y^T            U0 = exp(X0 + c0)      H0 = ln(1 + U0)
    X1 = W1z @ H0             U1 = exp(X1 + c1)      H1 = ln(1 + U1)
    dx^T = H1^T @ W2z^T + c2                         (batch-major, on PE)
    G2 = W2z^T @ e^T          GS1 = G2 * sigmoid(X1 + c1)
    G1 = W1z^T @ GS1          GS0 = G1 * sigmoid(X0 + c0)
    EJ = W0z^T @ GS0          div = sum_f EJ * e^T   (folded into PE as N=1 matmul)
    out = [dx, -div]

where  cI = t * WI[:, 0] + bI  and  WIz = WI[:, 1:]   (host-precomputed).

sigmoid(x) = U/(1+U) with U = exp(x+c) computed by ACT (Exp);  the product
G * U/(1+U) is ONE fused custom-DVE op (bit-trick reciprocal seed + 1 NR).
softplus is ACT Ln(U + 1) (Exp and Ln share one ACT table set -> no table
thrash; this build has no native Softplus table).

Input transposes are done by DMA (xbar) in bf16: y/e are cast fp32->bf16
DRAM->DRAM by SWDGE, then transpose-loaded viewing [rows,64] as [rows/2,128]
(the xbar needs >=128 source columns).  This interleaves batch parity on the
partition dim; all layouts below carry an explicit parity index.

Sharding: pure data parallel, batch split across 8 cores.
"""

import functools

import ml_dtypes
import numpy as np

import concourse.bass as bass
import concourse.mybir as mybir
import concourse.tile as tile
import concourse.dve_ops as dve_ops
from concourse import bacc
from concourse.dve_ops import DveOp
from concourse.dve_spec import C0, C1, C2, AluOp, Bin, Spec, Src0, Src1

P = 128
D = 64
H = 256
B_FULL = 262144
N_CORES = 8
B_CORE = B_FULL // N_CORES
SLAB = 4096          # batch rows per slab (DMA granularity)
PAIR = 1024          # batch rows per inner pipeline step
F32 = mybir.dt.float32
BF16 = mybir.dt.bfloat16
F16 = mybir.dt.float16   # H tiles / fwd weights: fp16 keeps 10 mantissa bits
                         # and enables FWL (2x faster LDWEIGHTS) on fwd2

# 1/(1+u) seed constants (Chebyshev pair over the bitcast(~t)*t interval,
# near-optimal for the 1-NR variant as well; max rel err ~1.7e-3).
SIG_C0 = -0.23549792
SIG_C1 = 2.0017324


# --------------------------------------------------------------------------
# fused  out = g * u / (1 + u)  custom DVE op (one instruction, 8 uop stages)
# --------------------------------------------------------------------------
def _make_sigmul_op() -> DveOp:
    name = "SIGMUL_EXP_ANT"
    for op in dve_ops.OPS:
        if op.name == name:
            return op
    _t = Src1 + C2
    _nt = Bin(AluOp.BITWISE_NOT, _t, _t)
    _y0 = _nt * C0
    _y1 = _y0 * (C1 - _t * _y0)

    def _ref(in0, in1, s0, s1, imm2):
        t = (in1.astype(np.float32) + np.float32(imm2)).astype(np.float32)
        nt = (~t.view(np.int32)).view(np.float32)
        y0 = (nt * np.float32(s0)).astype(np.float32)
        y1 = (y0 * (np.float32(s1) - t * y0)).astype(np.float32)
        return (in0.astype(np.float32) * in1 * y1).astype(np.float32)

    spec = Spec(body=(Src0 * Src1) * _y1, reference=_ref)
    op = DveOp(name, spec, subdim=False,
               uops_sha={"v3": "110f70764f90f15f"})
    # self-heal the sha pin if lower() output differs in this environment
    dve_ops.OPS.append(op)
    dve_ops._SUB_OPCODE_FOR_NAME[name] = max(
        dve_ops._SUB_OPCODE_FOR_NAME.values()) + 1
    dve_ops.CUSTOM_DVE_SPECS[name] = spec
    for ver in ("v3",):
        try:
            op.compile(ver)
        except ValueError as ex:
            got = str(ex).split('"')[-2]
            dve_ops._COMPILE_CACHE.pop((name, ver), None)
            object.__setattr__(op, "uops_sha", {**op.uops_sha, ver: got})
            op.compile(ver)
    return op


SIGMUL = _make_sigmul_op()


def _patch_act_tables():
    """Pin Exp and Ln to the one table set containing both
    (natural_log_exp_and_others) so bacc never thrashes ACT table loads
    between exp_and_others / natural_log (~2.7us per switch)."""
    import concourse.bacc as _bacc

    from concourse.hw_specs import get_activation_tables as _gat

    @functools.cache
    def patched(arch):
        exp = mybir.ActivationFunctionType.Exp
        ln = mybir.ActivationFunctionType.Ln
        out = {}
        for name, fns in _gat(arch).items():
            if name != "natural_log_exp_and_others":
                fns = fns - {exp, ln}
            out[name] = set(fns)
        return out

    _bacc.get_activation_tables = patched


_patch_act_tables()


# --------------------------------------------------------------------------
# program builder
# --------------------------------------------------------------------------
def build_program(b_core: int = B_CORE, slab: int = SLAB):
    assert b_core % slab == 0 and slab % PAIR == 0
    n_slabs = b_core // slab
    pairs_per_slab = slab // PAIR
    rcols_slab = slab // 2          # transpose-view columns per slab
    jblocks = slab // P             # 65-wide out-staging blocks per slab

    nc = bacc.Bacc("TRN2", target_bir_lowering=False, debug=False)

    y = nc.dram_tensor("y", [b_core, D + 1], F32, kind="ExternalInput")
    e = nc.dram_tensor("e", [b_core, D], F32, kind="ExternalInput")
    wf0 = nc.dram_tensor("wf0", [P, 2, 2, P], BF16, kind="ExternalInput")
    wb2 = nc.dram_tensor("wb2", [P, 2, 2, P], BF16, kind="ExternalInput")
    wf1 = nc.dram_tensor("wf1", [P, 2, H], F16, kind="ExternalInput")
    wb1 = nc.dram_tensor("wb1", [P, 2, H], F16, kind="ExternalInput")
    wb0 = nc.dram_tensor("wb0", [P, 2, D], F16, kind="ExternalInput")
    wf2 = nc.dram_tensor("wf2", [P, 2, D], F16, kind="ExternalInput")
    c0s = nc.dram_tensor("c0s", [P, 2], F32, kind="ExternalInput")
    c1s = nc.dram_tensor("c1s", [P, 2], F32, kind="ExternalInput")
    c2pat = nc.dram_tensor("c2pat", [P, 4 * (D + 1)], F32, kind="ExternalInput")
    negones = nc.dram_tensor("negones", [P, 1], F32, kind="ExternalInput")
    out = nc.dram_tensor("out", [b_core, D + 1], F32, kind="ExternalOutput")

    # out row = s*slab + (pr*4+g)*256 + p*2 + two; staging j = pr*8+g*2+two.
    # (pr,g) merge to one uniform-stride dim, (two,f) merge to 130 contiguous.
    out_v = out[:].rearrange(
        "(s prg p two) f -> s p prg (two f)",
        s=n_slabs, prg=4 * pairs_per_slab, p=P, two=2)

    exp_t = mybir.ActivationFunctionType.Exp
    ln_t = mybir.ActivationFunctionType.Ln

    with tile.TileContext(nc) as tc:
        with (
            tc.tile_pool(name="singles", bufs=1) as singles,
            tc.tile_pool(name="dram", bufs=2, space="DRAM") as dpool,
            tc.tile_pool(name="yt", bufs=2) as p_yt,
            tc.tile_pool(name="et", bufs=2) as p_et,
            tc.tile_pool(name="u", bufs=2) as p_u,
            tc.tile_pool(name="h", bufs=2) as p_h,
            tc.tile_pool(name="gs", bufs=2) as p_gs,
            tc.tile_pool(name="pp", bufs=2) as p_p,
            tc.tile_pool(name="outs", bufs=2) as p_out,
            tc.tile_pool(name="ps", bufs=4, space="PSUM") as ps,
        ):
            # ---- constants / weights into SBUF -------------------------
            sb_wf0 = singles.tile([P, 2, 2, P], BF16)
            sb_wb2 = singles.tile([P, 2, 2, P], BF16)
            sb_wf1 = singles.tile([P, 2, H], F16)
            sb_wb1 = singles.tile([P, 2, H], F16)
            sb_wb0 = singles.tile([P, 2, D], F16)
            sb_wf2 = singles.tile([P, 2, D], F16)
            sb_c0 = singles.tile([P, 2], F32)
            sb_c1 = singles.tile([P, 2], F32)
            sb_c2 = singles.tile([P, 4 * (D + 1)], F32)
            sb_neg = singles.tile([P, 1], F32)
            for sb, dr in ((sb_wf0, wf0), (sb_wb2, wb2), (sb_wf1, wf1),
                           (sb_wb1, wb1), (sb_wb0, wb0), (sb_wf2, wf2),
                           (sb_c0, c0s), (sb_c1, c1s), (sb_c2, c2pat),
                           (sb_neg, negones)):
                nc.sync.dma_start(out=sb[:], in_=dr[:])

            n_pairs = b_core // PAIR
            slab_res = {}

            def load_slab(s):
                if s in slab_res or s >= n_slabs:
                    return
                rows = slice(s * slab, (s + 1) * slab)
                # fp32 -> bf16 cast passes (SWDGE), DRAM -> DRAM
                ybf = dpool.tile([slab, D], BF16, tag="ybf")
                ebf = dpool.tile([slab, D], BF16, tag="ebf")
                with nc.allow_non_contiguous_dma(reason="256B row chunks"):
                    nc.gpsimd.dma_start(out=ybf[:], in_=y[rows, 0:D])
                nc.gpsimd.dma_start(out=ebf[:], in_=e[rows, :])
                # transpose loads: [slab/2, 128] -> [128, slab/2]
                # partition c<64: feature c of even rows; c>=64: odd rows
                yt = p_yt.tile([P, rcols_slab], BF16)
                et = p_et.tile([P, rcols_slab], BF16)
                nc.sync.dma_start_transpose(
                    yt[:], ybf[:].rearrange("(q t) f -> q (t f)", t=2))
                nc.sync.dma_start_transpose(
                    et[:], ebf[:].rearrange("(q t) f -> q (t f)", t=2))
                outs = p_out.tile([P, jblocks, D + 1], F32)
                slab_res[s] = (yt, et, outs)

            def rc_of(t):
                return slice((t % pairs_per_slab) * 512,
                             (t % pairs_per_slab + 1) * 512)

            def x0_mms(t):
                """X0 matmuls for pair t -> two 2-bank psum m-chunk tiles."""
                yt = slab_res[t // pairs_per_slab][0]
                rc = rc_of(t)
                x0m = []
                for m in range(2):
                    xm = ps.tile([P, 2, 512], F32, tag="ps")
                    for pi in range(2):
                        hs = slice(pi * D, (pi + 1) * D)
                        nc.tensor.matmul(
                            out=xm[:, pi, :],
                            lhsT=sb_wf0[hs, pi, m, :],
                            rhs=yt[hs, rc],
                            start=True, stop=True,
                            tile_position=(pi * D, 0))
                    x0m.append(xm)
                return x0m

            def exps(xm, bias_sb, utag):
                """U = Exp(X + c) per m-chunk."""
                u = p_u.tile([P, 2, 1024], F32, tag=utag)
                for m in range(2):
                    nc.scalar.activation(
                        out=u[:, m, :], in_=xm[m][:, :, :],
                        func=exp_t, bias=bias_sb[:, m:m + 1], scale=1.0)
                return u

            def ln1p(u, htag):
                """H = Ln(U + 1)."""
                h = p_h.tile([P, 2, 1024], F16, tag=htag)
                nc.scalar.activation(
                    out=h[:, :, :], in_=u[:, :, :],
                    func=ln_t, bias=1.0, scale=1.0)
                return h

            def x1_mms(t, h0):
                x1m = []
                for m in range(2):
                    xm = ps.tile([P, 2, 512], F32, tag="ps")
                    for k in range(2):
                        for pi in range(2):
                            nc.tensor.matmul(
                                out=xm[:, pi, :],
                                lhsT=sb_wf1[:, k, m * P:(m + 1) * P],
                                rhs=h0[:, k, pi * 512:(pi + 1) * 512],
                                start=(k == 0), stop=(k == 1))
                    x1m.append(xm)
                return x1m

            # ---- software-pipelined main loop (3 stages deep) ---------
            # iteration t: Exp1/Ln1 + backward of pair t, tail of pair
            # t-1, forward L0 + X1 matmuls of pair t+1.
            load_slab(0)
            load_slab(1)
            x0m = x0_mms(0)
            u0 = exps(x0m, sb_c0, "U0")
            h0 = ln1p(u0, "H0")
            x1m = x1_mms(0, h0)
            tailctx = None
            for t in range(n_pairs):
                s, pr = divmod(t, pairs_per_slab)
                if pr == 0:
                    load_slab(s + 2)
                et = slab_res[s][1]
                rc = rc_of(t)

                # 1. ACT: U1 of pair t (X1 matmuls ran last iteration)
                u1 = exps(x1m, sb_c1, "U1")

                # 2. G2 = W2z^T e^T (row-packed)
                g2m = []
                for m in range(2):
                    gm = ps.tile([P, 2, 512], F32, tag="ps")
                    for pi in range(2):
                        hs = slice(pi * D, (pi + 1) * D)
                        nc.tensor.matmul(
                            out=gm[:, pi, :],
                            lhsT=sb_wb2[hs, pi, m, :],
                            rhs=et[hs, rc],
                            start=True, stop=True,
                            tile_position=(pi * D, 0))
                    g2m.append(gm)

                # 3. forward L0 of pair t+1 (ACT: Exp0, Ln0 right after
                #    Exp1 so X1 matmuls of t+1 get a head start during Ln1)
                if t + 1 < n_pairs:
                    x0m = x0_mms(t + 1)
                    u0n = exps(x0m, sb_c0, "U0")
                    h0n = ln1p(u0n, "H0")

                # 4. GS1 = G2 * sigmoid(X1+c1)
                gs1 = p_gs.tile([P, 2, 1024], F16, tag="GS1")
                for m in range(2):
                    nc.vector._custom_dve(
                        SIGMUL,
                        out=gs1[:, m, :],
                        in0=g2m[m][:].rearrange("p a b -> p (a b)"),
                        in1=u1[:, m, :],
                        s0=SIG_C0, s1=SIG_C1, imm2=1.0)

                # 4+5. tail of pair t-1: EJ matmuls + P = EJ*e
                if tailctx is not None:
                    tp_, gs0_, h1_ = tailctx
                    et_ = slab_res[tp_ // pairs_per_slab][1]
                    ej = ps.tile([P, 512], F32, tag="ps")
                    for k in range(2):
                        for pi in range(2):
                            nc.tensor.matmul(
                                out=ej[pi * D:(pi + 1) * D, :],
                                lhsT=sb_wb0[:, k, :],
                                rhs=gs0_[:, k, pi * 512:(pi + 1) * 512],
                                start=(k == 0), stop=(k == 1),
                                tile_position=(0, pi * D),
                                skip_group_check=True)
                    pmul = p_p.tile([P, 512], F32)
                    nc.vector.tensor_mul(pmul[:], ej[:], et_[:, rc_of(tp_)])

                # 7. ACT: H1 = Ln(U1 + 1) (late: first use is next iter)
                h1 = ln1p(u1, "H1")

                # 8+9. tail of pair t-1: dx/div matmuls + staging add
                if tailctx is not None:
                    tp_, gs0_, h1_ = tailctx
                    sp_, pp_ = divmod(tp_, pairs_per_slab)
                    outs_ = slab_res[sp_][2]
                    dxt = ps.tile([P, 2, 512], F32, tag="ps")
                    for pi in range(2):
                        for g in range(4):
                            col = slice(pi * 512 + g * P,
                                        pi * 512 + (g + 1) * P)
                            o = g * (D + 1)
                            for k in range(2):
                                nc.tensor.matmul(
                                    out=dxt[:, pi, o:o + D],
                                    lhsT=h1_[:, k, col],
                                    rhs=sb_wf2[:, k, :],
                                    start=(k == 0), stop=(k == 1))
                    for g in range(4):
                        for pi in range(2):
                            o = g * (D + 1)
                            nc.tensor.matmul(
                                out=dxt[:, pi, o + D:o + D + 1],
                                lhsT=pmul[pi * D:(pi + 1) * D,
                                          g * P:(g + 1) * P],
                                rhs=sb_neg[pi * D:(pi + 1) * D, :],
                                start=True, stop=True,
                                tile_position=(pi * D, 0))
                    outs_v = outs_[:].rearrange(
                        "p (pr g two) f -> p pr g two f", g=4, two=2)
                    for pi in range(2):
                        nc.vector.tensor_add(
                            outs_v[:, pp_, :, pi, :],
                            dxt[:, pi, 0:4 * (D + 1)].rearrange(
                                "p (g f) -> p g f", f=D + 1),
                            sb_c2[:, :].rearrange(
                                "p (g f) -> p g f", f=D + 1))
                    if pp_ == pairs_per_slab - 1:
                        nc.sync.dma_start(
                            out=out_v[sp_],
                            in_=outs_[:].rearrange(
                                "p (prg two) f -> p prg (two f)", two=2))

                # 10. backward layer 1
                g1m = []
                for m in range(2):
                    gm = ps.tile([P, 2, 512], F32, tag="ps")
                    for k in range(2):
                        for pi in range(2):
                            nc.tensor.matmul(
                                out=gm[:, pi, :],
                                lhsT=sb_wb1[:, k, m * P:(m + 1) * P],
                                rhs=gs1[:, k, pi * 512:(pi + 1) * 512],
                                start=(k == 0), stop=(k == 1))
                    g1m.append(gm)

                # 11. GS0 = G1 * sigmoid(X0+c0)
                gs0 = p_gs.tile([P, 2, 1024], F16, tag="GS0")
                for m in range(2):
                    nc.vector._custom_dve(
                        SIGMUL,
                        out=gs0[:, m, :],
                        in0=g1m[m][:].rearrange("p a b -> p (a b)"),
                        in1=u0[:, m, :],
                        s0=SIG_C0, s1=SIG_C1, imm2=1.0)

                # 12. X1 matmuls of pair t+1
                tailctx = (t, gs0, h1)
                if t + 1 < n_pairs:
                    x1m = x1_mms(t + 1, h0n)
                    u0, h0 = u0n, h0n

            # ---- epilogue: tail of the last pair ----------------------
            tp_, gs0_, h1_ = tailctx
            et_ = slab_res[tp_ // pairs_per_slab][1]
            ej = ps.tile([P, 512], F32, tag="ps")
            for k in range(2):
                for pi in range(2):
                    nc.tensor.matmul(
                        out=ej[pi * D:(pi + 1) * D, :],
                        lhsT=sb_wb0[:, k, :],
                        rhs=gs0_[:, k, pi * 512:(pi + 1) * 512],
                        start=(k == 0), stop=(k == 1),
                        tile_position=(0, pi * D),
                        skip_group_check=True)
            pmul = p_p.tile([P, 512], F32)
            nc.vector.tensor_mul(pmul[:], ej[:], et_[:, rc_of(tp_)])
            sp_, pp_ = divmod(tp_, pairs_per_slab)
            outs_ = slab_res[sp_][2]
            dxt = ps.tile([P, 2, 512], F32, tag="ps")
            for pi in range(2):
                for g in range(4):
                    col = slice(pi * 512 + g * P, pi * 512 + (g + 1) * P)
                    o = g * (D + 1)
                    for k in range(2):
                        nc.tensor.matmul(
                            out=dxt[:, pi, o:o + D],
                            lhsT=h1_[:, k, col],
                            rhs=sb_wf2[:, k, :],
                            start=(k == 0), stop=(k == 1))
            for g in range(4):
                for pi in range(2):
                    o = g * (D + 1)
                    nc.tensor.matmul(
                        out=dxt[:, pi, o + D:o + D + 1],
                        lhsT=pmul[pi * D:(pi + 1) * D, g * P:(g + 1) * P],
                        rhs=sb_neg[pi * D:(pi + 1) * D, :],
                        start=True, stop=True,
                        tile_position=(pi * D, 0))
            outs_v = outs_[:].rearrange(
                "p (pr g two) f -> p pr g two f", g=4, two=2)
            for pi in range(2):
                nc.vector.tensor_add(
                    outs_v[:, pp_, :, pi, :],
                    dxt[:, pi, 0:4 * (D + 1)].rearrange(
                        "p (g f) -> p g f", f=D + 1),
                    sb_c2[:, :].rearrange("p (g f) -> p g f", f=D + 1))
            nc.sync.dma_start(
                out=out_v[sp_],
                in_=outs_[:].rearrange(
                    "p (prg two) f -> p prg (two f)", two=2))

    nc.compile()
    return nc


# --------------------------------------------------------------------------
# host-side weight preparation
# --------------------------------------------------------------------------
def prepare_consts(t, W0, b0, W1, b1, W2, b2):
    f32 = np.float32
    bf = ml_dtypes.bfloat16
    tval = f32(t[0])
    W0z = W0[:, 1:].astype(f32)   # [H, D]
    W1z = W1[:, 1:].astype(f32)   # [H, H]
    W2z = W2[:, 1:].astype(f32)   # [D, H]
    c0 = (tval * W0[:, 0] + b0).astype(f32)
    c1 = (tval * W1[:, 0] + b1).astype(f32)
    c2 = (tval * W2[:, 0] + b2).astype(f32)

    # wf0[p, pi, m, j]: even rows: W0z[m*128+j, p]; odd: shifted down 64
    wf0 = np.zeros((P, 2, 2, P), f32)
    w0zT = W0z.T                                  # [D, H]
    for m in range(2):
        blk = w0zT[:, m * P:(m + 1) * P]          # [64, 128]
        wf0[0:D, 0, m, :] = blk
        wf0[D:P, 1, m, :] = blk
    # wb2: lhsT = W2z rows (k=feature of e), out = G2 features
    wb2 = np.zeros((P, 2, 2, P), f32)
    for m in range(2):
        blk = W2z[:, m * P:(m + 1) * P]           # [64, 128]
        wb2[0:D, 0, m, :] = blk
        wb2[D:P, 1, m, :] = blk
    # wf1[p, k, m*128+c] = W1zT[k*128+p, m*128+c] = W1z[m*128+c, k*128+p]
    w1zT = W1z.T
    wf1 = np.stack([w1zT[k * P:(k + 1) * P, :] for k in range(2)], axis=1)
    # wb1[p, k, m*128+c] = W1z[k*128+p, m*128+c]
    wb1 = np.stack([W1z[k * P:(k + 1) * P, :] for k in range(2)], axis=1)
    # wb0[p, k, j] = W0z[k*128+p, j]
    wb0 = np.stack([W0z[k * P:(k + 1) * P, :] for k in range(2)], axis=1)
    # wf2[p, k, j] = W2zT[k*128+p, j] = W2z[j, k*128+p]
    w2zT = W2z.T
    wf2 = np.stack([w2zT[k * P:(k + 1) * P, :] for k in range(2)], axis=1)

    c0s = c0.reshape(2, P).T.copy()               # [128, 2]
    c1s = c1.reshape(2, P).T.copy()
    c2pat = np.zeros((P, 4 * (D + 1)), f32)
    row = np.concatenate([c2, [0.0]]).astype(f32)
    c2pat[:, :] = np.tile(row, 4)[None, :]
    neg = np.full((P, 1), -1.0, f32)

    return {
        "wf0": np.ascontiguousarray(wf0.astype(bf)),
        "wb2": np.ascontiguousarray(wb2.astype(bf)),
        "wf1": np.ascontiguousarray(wf1.astype(np.float16)),
        "wb1": np.ascontiguousarray(wb1.astype(np.float16)),
        "wb0": np.ascontiguousarray(wb0.astype(np.float16)),
        "wf2": np.ascontiguousarray(wf2.astype(np.float16)),
        "c0s": np.ascontiguousarray(c0s),
        "c1s": np.ascontiguousarray(c1s),
        "c2pat": c2pat,
        "negones": neg,
    }


@functools.lru_cache(maxsize=1)
def _get_program():
    return build_program(B_CORE, SLAB)


def kernel(t, y, e, W0, b0, W1, b1, W2, b2):
    from concourse.bass_utils import run_bass_kernel_spmd

    consts = prepare_consts(t, W0, b0, W1, b1, W2, b2)
    nc = _get_program()

    in_maps = []
    for c in range(N_CORES):
        rows = slice(c * B_CORE, (c + 1) * B_CORE)
        m = {"y": np.ascontiguousarray(y[rows].astype(np.float32)),
             "e": np.ascontiguousarray(e[rows].astype(np.float32))}
        m.update(consts)
        in_maps.append(m)

    res = run_bass_kernel_spmd(nc, in_maps, core_ids=list(range(N_CORES)))
    return np.concatenate([r["out"] for r in res.results], axis=0)


# revision 29
# speedup vs baseline: 6630.2603x; 6630.2603x over previous
"""Trainium2 Bass kernel for the FFJORD-style ODE function
(3-layer ConcatLinear MLP with softplus + Hutchinson divergence via VJP).

Math (per batch row, feature-major on device):
    X0 = W0z @ y^T            U0 = exp(X0 + c0)      H0 = ln(1 + U0)
    X1 = W1z @ H0             U1 = exp(X1 + c1)      H1 = ln(1 + U1)
    dx^T = H1^T @ W2z^T + c2                         (batch-major, on PE)
    G2 = W2z^T @ e^T          GS1 = G2 * sigmoid(X1 + c1)
    G1 = W1z^T @ GS1          GS0 = G1 * sigmoid(X0 + c0)
    EJ = W0z^T @ GS0          div = sum_f EJ * e^T   (folded into PE as N=1 matmul)
    out = [dx, -div]

where  cI = t * WI[:, 0] + bI  and  WIz = WI[:, 1:]   (host-precomputed).

sigmoid(x) = U/(1+U) with U = exp(x+c) computed by ACT (Exp);  the product
G * U/(1+U) is ONE fused custom-DVE op (bit-trick reciprocal seed + 1 NR).
softplus is ACT Ln(U + 1) (Exp and Ln share one ACT table set -> no table
thrash; this build has no native Softplus table).

Input transposes are done by DMA (xbar) in bf16: y/e are cast fp32->bf16
DRAM->DRAM by SWDGE, then transpose-loaded viewing [rows,64] as [rows/2,128]
(the xbar needs >=128 source columns).  This interleaves batch parity on the
partition dim; all layouts below carry an explicit parity index.

Sharding: pure data parallel, batch split across 8 cores.
"""

import functools

import ml_dtypes
import numpy as np

import concourse.bass as bass
import concourse.mybir as mybir
import concourse.tile as tile
import concourse.dve_ops as dve_ops
from concourse import bacc
from concourse.dve_ops import DveOp
from concourse.dve_spec import C0, C1, C2, AluOp, Bin, Spec, Src0, Src1

P = 128
D = 64
H = 256
B_FULL = 262144
N_CORES = 8
B_CORE = B_FULL // N_CORES
SLAB = 4096          # batch rows per slab (DMA granularity)
PAIR = 1024          # batch rows per inner pipeline step
F32 = mybir.dt.float32
BF16 = mybir.dt.bfloat16
F16 = mybir.dt.float16   # H tiles / fwd weights: fp16 keeps 10 mantissa bits
                         # and enables FWL (2x faster LDWEIGHTS) on fwd2

# 1/(1+u) seed constants (Chebyshev pair over the bitcast(~t)*t interval,
# near-optimal for the 1-NR variant as well; max rel err ~1.7e-3).
SIG_C0 = -0.23549792
SIG_C1 = 2.0017324


# --------------------------------------------------------------------------
# fused  out = g * u / (1 + u)  custom DVE op (one instruction, 8 uop stages)
# --------------------------------------------------------------------------
def _make_sigmul_op() -> DveOp:
    name = "SIGMUL_EXP_ANT"
    for op in dve_ops.OPS:
        if op.name == name:
            return op
    _t = Src1 + C2
    _nt = Bin(AluOp.BITWISE_NOT, _t, _t)
    _y0 = _nt * C0
    _y1 = _y0 * (C1 - _t * _y0)

    def _ref(in0, in1, s0, s1, imm2):
        t = (in1.astype(np.float32) + np.float32(imm2)).astype(np.float32)
        nt = (~t.view(np.int32)).view(np.float32)
        y0 = (nt * np.float32(s0)).astype(np.float32)
        y1 = (y0 * (np.float32(s1) - t * y0)).astype(np.float32)
        return (in0.astype(np.float32) * in1 * y1).astype(np.float32)

    spec = Spec(body=(Src0 * Src1) * _y1, reference=_ref)
    op = DveOp(name, spec, subdim=False,
               uops_sha={"v3": "110f70764f90f15f"})
    # self-heal the sha pin if lower() output differs in this environment
    dve_ops.OPS.append(op)
    dve_ops._SUB_OPCODE_FOR_NAME[name] = max(
        dve_ops._SUB_OPCODE_FOR_NAME.values()) + 1
    dve_ops.CUSTOM_DVE_SPECS[name] = spec
    for ver in ("v3",):
        try:
            op.compile(ver)
        except ValueError as ex:
            got = str(ex).split('"')[-2]
            dve_ops._COMPILE_CACHE.pop((name, ver), None)
            object.__setattr__(op, "uops_sha", {**op.uops_sha, ver: got})
            op.compile(ver)
    return op


SIGMUL = _make_sigmul_op()


def _patch_act_tables():
    """Pin Exp and Ln to the one table set containing both
    (natural_log_exp_and_others) so bacc never thrashes ACT table loads
    between exp_and_others / natural_log (~2.7us per switch)."""
    import concourse.bacc as _bacc

    from concourse.hw_specs import get_activation_tables as _gat

    @functools.cache
    def patched(arch):
        exp = mybir.ActivationFunctionType.Exp
        ln = mybir.ActivationFunctionType.Ln
        out = {}
        for name, fns in _gat(arch).items():
            if name != "natural_log_exp_and_others":
                fns = fns - {exp, ln}
            out[name] = set(fns)
        return out

    _bacc.get_activation_tables = patched


_patch_act_tables()


# --------------------------------------------------------------------------
# program builder
# --------------------------------------------------------------------------
def build_program(b_core: int = B_CORE, slab: int = SLAB):
    assert b_core % slab == 0 and slab % PAIR == 0
    n_slabs = b_core // slab
    pairs_per_slab = slab // PAIR
    rcols_slab = slab // 2          # transpose-view columns per slab
    jblocks = slab // P             # 65-wide out-staging blocks per slab

    nc = bacc.Bacc("TRN2", target_bir_lowering=False, debug=False)

    y = nc.dram_tensor("y", [b_core, D + 1], F32, kind="ExternalInput")
    e = nc.dram_tensor("e", [b_core, D], F32, kind="ExternalInput")
    wf0 = nc.dram_tensor("wf0", [P, 2, 2, P], BF16, kind="ExternalInput")
    wb2 = nc.dram_tensor("wb2", [P, 2, 2, P], BF16, kind="ExternalInput")
    wf1 = nc.dram_tensor("wf1", [P, 2, H], F16, kind="ExternalInput")
    wb1 = nc.dram_tensor("wb1", [P, 2, H], F16, kind="ExternalInput")
    wb0 = nc.dram_tensor("wb0", [P, 2, D], F16, kind="ExternalInput")
    wf2 = nc.dram_tensor("wf2", [P, 2, D], F16, kind="ExternalInput")
    c0s = nc.dram_tensor("c0s", [P, 2], F32, kind="ExternalInput")
    c1s = nc.dram_tensor("c1s", [P, 2], F32, kind="ExternalInput")
    c2pat = nc.dram_tensor("c2pat", [P, 4 * (D + 1)], F32, kind="ExternalInput")
    negones = nc.dram_tensor("negones", [P, 1], F16, kind="ExternalInput")
    out = nc.dram_tensor("out", [b_core, D + 1], F32, kind="ExternalOutput")

    # out row = s*slab + (pr*4+g)*256 + p*2 + two; staging j = pr*8+g*2+two.
    # (pr,g) merge to one uniform-stride dim, (two,f) merge to 130 contiguous.
    out_v = out[:].rearrange(
        "(s prg p two) f -> s p prg (two f)",
        s=n_slabs, prg=4 * pairs_per_slab, p=P, two=2)

    exp_t = mybir.ActivationFunctionType.Exp
    ln_t = mybir.ActivationFunctionType.Ln

    with tile.TileContext(nc) as tc:
        with (
            tc.tile_pool(name="singles", bufs=1) as singles,
            tc.tile_pool(name="dram", bufs=2, space="DRAM") as dpool,
            tc.tile_pool(name="yt", bufs=2) as p_yt,
            tc.tile_pool(name="et", bufs=2) as p_et,
            tc.tile_pool(name="u", bufs=2) as p_u,
            tc.tile_pool(name="h", bufs=2) as p_h,
            tc.tile_pool(name="gs", bufs=2) as p_gs,
            tc.tile_pool(name="pp", bufs=2) as p_p,
            tc.tile_pool(name="outs", bufs=2) as p_out,
            tc.tile_pool(name="ps", bufs=4, space="PSUM") as ps,
        ):
            # ---- constants / weights into SBUF -------------------------
            sb_wf0 = singles.tile([P, 2, 2, P], BF16)
            sb_wb2 = singles.tile([P, 2, 2, P], BF16)
            sb_wf1 = singles.tile([P, 2, H], F16)
            sb_wb1 = singles.tile([P, 2, H], F16)
            sb_wb0 = singles.tile([P, 2, D], F16)
            sb_wf2 = singles.tile([P, 2, D], F16)
            sb_c0 = singles.tile([P, 2], F32)
            sb_c1 = singles.tile([P, 2], F32)
            sb_c2 = singles.tile([P, 4 * (D + 1)], F32)
            sb_neg = singles.tile([P, 1], F16)
            for sb, dr in ((sb_wf0, wf0), (sb_wb2, wb2), (sb_wf1, wf1),
                           (sb_wb1, wb1), (sb_wb0, wb0), (sb_wf2, wf2),
                           (sb_c0, c0s), (sb_c1, c1s), (sb_c2, c2pat),
                           (sb_neg, negones)):
                nc.sync.dma_start(out=sb[:], in_=dr[:])

            n_pairs = b_core // PAIR
            slab_res = {}

            def load_slab(s):
                if s in slab_res or s >= n_slabs:
                    return
                rows = slice(s * slab, (s + 1) * slab)
                # fp32 -> bf16 cast passes (SWDGE), DRAM -> DRAM
                ybf = dpool.tile([slab, D], BF16, tag="ybf")
                ebf = dpool.tile([slab, D], BF16, tag="ebf")
                with nc.allow_non_contiguous_dma(reason="256B row chunks"):
                    nc.gpsimd.dma_start(out=ybf[:], in_=y[rows, 0:D])
                nc.gpsimd.dma_start(out=ebf[:], in_=e[rows, :])
                # transpose loads: [slab/2, 128] -> [128, slab/2]
                # partition c<64: feature c of even rows; c>=64: odd rows
                yt = p_yt.tile([P, rcols_slab], BF16)
                et = p_et.tile([P, rcols_slab], BF16)
                nc.sync.dma_start_transpose(
                    yt[:], ybf[:].rearrange("(q t) f -> q (t f)", t=2))
                nc.sync.dma_start_transpose(
                    et[:], ebf[:].rearrange("(q t) f -> q (t f)", t=2))
                outs = p_out.tile([P, jblocks, D + 1], F32)
                slab_res[s] = (yt, et, outs)

            def rc_of(t):
                return slice((t % pairs_per_slab) * 512,
                             (t % pairs_per_slab + 1) * 512)

            def x0_mms(t):
                """X0 matmuls for pair t -> two 2-bank psum m-chunk tiles."""
                yt = slab_res[t // pairs_per_slab][0]
                rc = rc_of(t)
                x0m = []
                for m in range(2):
                    xm = ps.tile([P, 2, 512], F32, tag="ps")
                    for pi in range(2):
                        hs = slice(pi * D, (pi + 1) * D)
                        nc.tensor.matmul(
                            out=xm[:, pi, :],
                            lhsT=sb_wf0[hs, pi, m, :],
                            rhs=yt[hs, rc],
                            start=True, stop=True,
                            tile_position=(pi * D, 0))
                    x0m.append(xm)
                return x0m

            def exps(xm, bias_sb, utag):
                """U = Exp(X + c) per m-chunk."""
                u = p_u.tile([P, 2, 1024], F32, tag=utag)
                for m in range(2):
                    nc.scalar.activation(
                        out=u[:, m, :], in_=xm[m][:, :, :],
                        func=exp_t, bias=bias_sb[:, m:m + 1], scale=1.0)
                return u

            def ln1p(u, htag):
                """H = Ln(U + 1)."""
                h = p_h.tile([P, 2, 1024], F16, tag=htag)
                nc.scalar.activation(
                    out=h[:, :, :], in_=u[:, :, :],
                    func=ln_t, bias=1.0, scale=1.0)
                return h

            def x1_mms(t, h0):
                x1m = []
                for m in range(2):
                    xm = ps.tile([P, 2, 512], F32, tag="ps")
                    for k in range(2):
                        for pi in range(2):
                            nc.tensor.matmul(
                                out=xm[:, pi, :],
                                lhsT=sb_wf1[:, k, m * P:(m + 1) * P],
                                rhs=h0[:, k, pi * 512:(pi + 1) * 512],
                                start=(k == 0), stop=(k == 1))
                    x1m.append(xm)
                return x1m

            # ---- software-pipelined main loop (3 stages deep) ---------
            # iteration t: Exp1/Ln1 + backward of pair t, tail of pair
            # t-1, forward L0 + X1 matmuls of pair t+1.
            load_slab(0)
            load_slab(1)
            x0m = x0_mms(0)
            u0 = exps(x0m, sb_c0, "U0")
            h0 = ln1p(u0, "H0")
            x1m = x1_mms(0, h0)
            tailctx = None
            for t in range(n_pairs):
                s, pr = divmod(t, pairs_per_slab)
                if pr == 0:
                    load_slab(s + 2)
                et = slab_res[s][1]
                rc = rc_of(t)

                # 1. ACT: U1 of pair t (X1 matmuls ran last iteration)
                u1 = exps(x1m, sb_c1, "U1")

                # 2. G2 = W2z^T e^T (row-packed)
                g2m = []
                for m in range(2):
                    gm = ps.tile([P, 2, 512], F32, tag="ps")
                    for pi in range(2):
                        hs = slice(pi * D, (pi + 1) * D)
                        nc.tensor.matmul(
                            out=gm[:, pi, :],
                            lhsT=sb_wb2[hs, pi, m, :],
                            rhs=et[hs, rc],
                            start=True, stop=True,
                            tile_position=(pi * D, 0))
                    g2m.append(gm)

                # 3. forward L0 of pair t+1 (ACT: Exp0, Ln0 right after
                #    Exp1 so X1 matmuls of t+1 get a head start during Ln1)
                if t + 1 < n_pairs:
                    x0m = x0_mms(t + 1)
                    u0n = exps(x0m, sb_c0, "U0")
                    h0n = ln1p(u0n, "H0")

                # 4. GS1 = G2 * sigmoid(X1+c1)
                gs1 = p_gs.tile([P, 2, 1024], F16, tag="GS1")
                for m in range(2):
                    nc.vector._custom_dve(
                        SIGMUL,
                        out=gs1[:, m, :],
                        in0=g2m[m][:].rearrange("p a b -> p (a b)"),
                        in1=u1[:, m, :],
                        s0=SIG_C0, s1=SIG_C1, imm2=1.0)

                # 4+5. tail of pair t-1: EJ matmuls + P = EJ*e
                if tailctx is not None:
                    tp_, gs0_, h1_ = tailctx
                    et_ = slab_res[tp_ // pairs_per_slab][1]
                    ej = ps.tile([P, 512], F32, tag="ps")
                    for k in range(2):
                        for pi in range(2):
                            nc.tensor.matmul(
                                out=ej[pi * D:(pi + 1) * D, :],
                                lhsT=sb_wb0[:, k, :],
                                rhs=gs0_[:, k, pi * 512:(pi + 1) * 512],
                                start=(k == 0), stop=(k == 1),
                                tile_position=(0, pi * D),
                                skip_group_check=True)
                    pmul = p_p.tile([P, 512], F16)
                    nc.vector.tensor_mul(pmul[:], ej[:], et_[:, rc_of(tp_)])

                # 7. ACT: H1 = Ln(U1 + 1) (late: first use is next iter)
                h1 = ln1p(u1, "H1")

                # 8+9. tail of pair t-1: dx/div matmuls + staging add
                if tailctx is not None:
                    tp_, gs0_, h1_ = tailctx
                    sp_, pp_ = divmod(tp_, pairs_per_slab)
                    outs_ = slab_res[sp_][2]
                    dxt = ps.tile([P, 2, 512], F32, tag="ps")
                    for pi in range(2):
                        for g in range(4):
                            col = slice(pi * 512 + g * P,
                                        pi * 512 + (g + 1) * P)
                            o = g * (D + 1)
                            for k in range(2):
                                nc.tensor.matmul(
                                    out=dxt[:, pi, o:o + D],
                                    lhsT=h1_[:, k, col],
                                    rhs=sb_wf2[:, k, :],
                                    start=(k == 0), stop=(k == 1))
                    for g in range(4):
                        for pi in range(2):
                            o = g * (D + 1)
                            nc.tensor.matmul(
                                out=dxt[:, pi, o + D:o + D + 1],
                                lhsT=pmul[pi * D:(pi + 1) * D,
                                          g * P:(g + 1) * P],
                                rhs=sb_neg[pi * D:(pi + 1) * D, :],
                                start=True, stop=True,
                                tile_position=(pi * D, 0))
                    outs_v = outs_[:].rearrange(
                        "p (pr g two) f -> p pr g two f", g=4, two=2)
                    for pi in range(2):
                        nc.vector.tensor_add(
                            outs_v[:, pp_, :, pi, :],
                            dxt[:, pi, 0:4 * (D + 1)].rearrange(
                                "p (g f) -> p g f", f=D + 1),
                            sb_c2[:, :].rearrange(
                                "p (g f) -> p g f", f=D + 1))
                    if pp_ == pairs_per_slab - 1:
                        nc.sync.dma_start(
                            out=out_v[sp_],
                            in_=outs_[:].rearrange(
                                "p (prg two) f -> p prg (two f)", two=2))

                # 10. backward layer 1
                g1m = []
                for m in range(2):
                    gm = ps.tile([P, 2, 512], F32, tag="ps")
                    for k in range(2):
                        for pi in range(2):
                            nc.tensor.matmul(
                                out=gm[:, pi, :],
                                lhsT=sb_wb1[:, k, m * P:(m + 1) * P],
                                rhs=gs1[:, k, pi * 512:(pi + 1) * 512],
                                start=(k == 0), stop=(k == 1))
                    g1m.append(gm)

                # 11. GS0 = G1 * sigmoid(X0+c0)
                gs0 = p_gs.tile([P, 2, 1024], F16, tag="GS0")
                for m in range(2):
                    nc.vector._custom_dve(
                        SIGMUL,
                        out=gs0[:, m, :],
                        in0=g1m[m][:].rearrange("p a b -> p (a b)"),
                        in1=u0[:, m, :],
                        s0=SIG_C0, s1=SIG_C1, imm2=1.0)

                # 12. X1 matmuls of pair t+1
                tailctx = (t, gs0, h1)
                if t + 1 < n_pairs:
                    x1m = x1_mms(t + 1, h0n)
                    u0, h0 = u0n, h0n

            # ---- epilogue: tail of the last pair ----------------------
            tp_, gs0_, h1_ = tailctx
            et_ = slab_res[tp_ // pairs_per_slab][1]
            ej = ps.tile([P, 512], F32, tag="ps")
            for k in range(2):
                for pi in range(2):
                    nc.tensor.matmul(
                        out=ej[pi * D:(pi + 1) * D, :],
                        lhsT=sb_wb0[:, k, :],
                        rhs=gs0_[:, k, pi * 512:(pi + 1) * 512],
                        start=(k == 0), stop=(k == 1),
                        tile_position=(0, pi * D),
                        skip_group_check=True)
            pmul = p_p.tile([P, 512], F16)
            nc.vector.tensor_mul(pmul[:], ej[:], et_[:, rc_of(tp_)])
            sp_, pp_ = divmod(tp_, pairs_per_slab)
            outs_ = slab_res[sp_][2]
            dxt = ps.tile([P, 2, 512], F32, tag="ps")
            for pi in range(2):
                for g in range(4):
                    col = slice(pi * 512 + g * P, pi * 512 + (g + 1) * P)
                    o = g * (D + 1)
                    for k in range(2):
                        nc.tensor.matmul(
                            out=dxt[:, pi, o:o + D],
                            lhsT=h1_[:, k, col],
                            rhs=sb_wf2[:, k, :],
                            start=(k == 0), stop=(k == 1))
            for g in range(4):
                for pi in range(2):
                    o = g * (D + 1)
                    nc.tensor.matmul(
                        out=dxt[:, pi, o + D:o + D + 1],
                        lhsT=pmul[pi * D:(pi + 1) * D, g * P:(g + 1) * P],
                        rhs=sb_neg[pi * D:(pi + 1) * D, :],
                        start=True, stop=True,
                        tile_position=(pi * D, 0))
            outs_v = outs_[:].rearrange(
                "p (pr g two) f -> p pr g two f", g=4, two=2)
            for pi in range(2):
                nc.vector.tensor_add(
                    outs_v[:, pp_, :, pi, :],
                    dxt[:, pi, 0:4 * (D + 1)].rearrange(
                        "p (g f) -> p g f", f=D + 1),
                    sb_c2[:, :].rearrange("p (g f) -> p g f", f=D + 1))
            nc.sync.dma_start(
                out=out_v[sp_],
                in_=outs_[:].rearrange(
                    "p (prg two) f -> p prg (two f)", two=2))

    nc.compile()
    return nc


# --------------------------------------------------------------------------
# host-side weight preparation
# --------------------------------------------------------------------------
def prepare_consts(t, W0, b0, W1, b1, W2, b2):
    f32 = np.float32
    bf = ml_dtypes.bfloat16
    tval = f32(t[0])
    W0z = W0[:, 1:].astype(f32)   # [H, D]
    W1z = W1[:, 1:].astype(f32)   # [H, H]
    W2z = W2[:, 1:].astype(f32)   # [D, H]
    c0 = (tval * W0[:, 0] + b0).astype(f32)
    c1 = (tval * W1[:, 0] + b1).astype(f32)
    c2 = (tval * W2[:, 0] + b2).astype(f32)

    # wf0[p, pi, m, j]: even rows: W0z[m*128+j, p]; odd: shifted down 64
    wf0 = np.zeros((P, 2, 2, P), f32)
    w0zT = W0z.T                                  # [D, H]
    for m in range(2):
        blk = w0zT[:, m * P:(m + 1) * P]          # [64, 128]
        wf0[0:D, 0, m, :] = blk
        wf0[D:P, 1, m, :] = blk
    # wb2: lhsT = W2z rows (k=feature of e), out = G2 features
    wb2 = np.zeros((P, 2, 2, P), f32)
    for m in range(2):
        blk = W2z[:, m * P:(m + 1) * P]           # [64, 128]
        wb2[0:D, 0, m, :] = blk
        wb2[D:P, 1, m, :] = blk
    # wf1[p, k, m*128+c] = W1zT[k*128+p, m*128+c] = W1z[m*128+c, k*128+p]
    w1zT = W1z.T
    wf1 = np.stack([w1zT[k * P:(k + 1) * P, :] for k in range(2)], axis=1)
    # wb1[p, k, m*128+c] = W1z[k*128+p, m*128+c]
    wb1 = np.stack([W1z[k * P:(k + 1) * P, :] for k in range(2)], axis=1)
    # wb0[p, k, j] = W0z[k*128+p, j]
    wb0 = np.stack([W0z[k * P:(k + 1) * P, :] for k in range(2)], axis=1)
    # wf2[p, k, j] = W2zT[k*128+p, j] = W2z[j, k*128+p]
    w2zT = W2z.T
    wf2 = np.stack([w2zT[k * P:(k + 1) * P, :] for k in range(2)], axis=1)

    c0s = c0.reshape(2, P).T.copy()               # [128, 2]
    c1s = c1.reshape(2, P).T.copy()
    c2pat = np.zeros((P, 4 * (D + 1)), f32)
    row = np.concatenate([c2, [0.0]]).astype(f32)
    c2pat[:, :] = np.tile(row, 4)[None, :]
    neg = np.full((P, 1), -1.0, np.float16)

    return {
        "wf0": np.ascontiguousarray(wf0.astype(bf)),
        "wb2": np.ascontiguousarray(wb2.astype(bf)),
        "wf1": np.ascontiguousarray(wf1.astype(np.float16)),
        "wb1": np.ascontiguousarray(wb1.astype(np.float16)),
        "wb0": np.ascontiguousarray(wb0.astype(np.float16)),
        "wf2": np.ascontiguousarray(wf2.astype(np.float16)),
        "c0s": np.ascontiguousarray(c0s),
        "c1s": np.ascontiguousarray(c1s),
        "c2pat": c2pat,
        "negones": neg,
    }


@functools.lru_cache(maxsize=1)
def _get_program():
    return build_program(B_CORE, SLAB)


def kernel(t, y, e, W0, b0, W1, b1, W2, b2):
    from concourse.bass_utils import run_bass_kernel_spmd

    consts = prepare_consts(t, W0, b0, W1, b1, W2, b2)
    nc = _get_program()

    in_maps = []
    for c in range(N_CORES):
        rows = slice(c * B_CORE, (c + 1) * B_CORE)
        m = {"y": np.ascontiguousarray(y[rows].astype(np.float32)),
             "e": np.ascontiguousarray(e[rows].astype(np.float32))}
        m.update(consts)
        in_maps.append(m)

    res = run_bass_kernel_spmd(nc, in_maps, core_ids=list(range(N_CORES)))
    return np.concatenate([r["out"] for r in res.results], axis=0)
